# revision 1
# baseline (speedup 1.0000x reference)
"""Multi-head causal attention (B=2, L=2048, E=1024, H=16, D=64) on 8 NeuronCores.

Sharding: data-parallel over batch x tensor-parallel over heads.
  core c: batch b = c // 4, head group hg = c % 4 -> heads [4*hg, 4*hg+4).
Each core computes QKV projection for its 4 heads, causal softmax attention,
and a *partial* output projection (its heads' slice of Wout). The host sums
the 4 partial outputs per batch (bias is folded into core hg==0's partial).

Device notes:
  - Matmul operands are bf16 (fp32 PSUM accumulation); host pre-casts inputs.
  - Host pre-transposes everything so the device never transposes:
      xT   [E, L]   = x[b].T                      (bf16)
      waT  [E, 768] = Wa rows regrouped [q_h0..q_h3 | k_h0.. | v_h0..].T (bf16)
      woT  [256, E] = Wout_w columns for this core's heads, transposed   (bf16)
      bias [128, 8] = Wout_b per output-partition chunk (zeros unless hg==0)
  - Attention runs in the S^T layout (scores[j, i]); softmax denominator Z
    comes from a ones-column appended to V (PSUM row 64; row 65 is pad).
  - No max-subtraction in softmax: scores are ~N(0, 0.41^2), exp can't overflow.
  - 1/Z computed as exp(-ln(Z)) on ScalarE (DVE reciprocal is ~3.3us/op).
"""

import ml_dtypes
import numpy as np

import concourse.bass as bass
import concourse.mybir as mybir
import concourse.tile as tile
from concourse import bacc
from concourse import bass_utils as _bass_utils
from concourse.bass_utils import run_bass_kernel_spmd
from concourse.masks import make_upper_triangular


P = 128
B = 2
L = 2048
E = 1024
H = 16
D = 64
HC = 4            # heads per core
F = HC * D        # 256: this core's slice of the head dim
EC = E // P       # 8 chunks of the embed dim
NLC = L // P      # 16 l-chunks
VST = NLC * 66    # v stride per head: 16 chunks of [64 v | 1 ones | 1 pad]

f32 = mybir.dt.float32
f32r = mybir.dt.float32r
bf16 = mybir.dt.bfloat16
AF = mybir.ActivationFunctionType
N_CORES = 8


def build_nc():
    nc = bacc.Bacc(None, target_bir_lowering=False, debug=False)

    xT = nc.dram_tensor("xT", [E, L], bf16, kind="ExternalInput")
    waT = nc.dram_tensor("waT", [E, 3 * F], bf16, kind="ExternalInput")
    woT = nc.dram_tensor("woT", [F, E], bf16, kind="ExternalInput")
    bias = nc.dram_tensor("bias", [P, E // P], f32, kind="ExternalInput")
    outT = nc.dram_tensor("outT", [E, L], f32, kind="ExternalOutput")

    with tile.TileContext(nc) as tc:
        with (
            tc.tile_pool(name="persist", bufs=1) as pp,
            tc.tile_pool(name="qkv", bufs=1) as qp,
        ):
            # Persistent SBUF tensors.
            qT = [qp.tile([P, L], bf16, tag=f"q{p}", name=f"qT{p}") for p in range(2)]
            kT = [qp.tile([P, L], bf16, tag=f"k{p}", name=f"kT{p}") for p in range(2)]
            von = qp.tile([P, HC * VST], bf16, tag="von", name="von")
            oT = [qp.tile([P, L], bf16, tag=f"o{p}", name=f"oT{p}") for p in range(2)]
            wo_sb = [
                pp.tile([P, E], bf16, tag=f"wo{fc}", name=f"wo{fc}") for fc in range(2)
            ]
            bias_sb = pp.tile([P, E // P], f32, tag="bias")
            ones33 = pp.tile([33, 64], f32r, tag="ones33")
            trimask = pp.tile([P, P], bf16, tag="trimask")
            onesf = pp.tile([P, 64], f32, tag="onesf")
            trimaskf = pp.tile([P, P], f32, tag="trimaskf")
            zc = pp.tile([33, 1024], f32, tag="zc")
            zr = pp.tile([33, 1024], f32r, tag="zr")
            nc.gpsimd.memset(zc[:], 1.0)  # dead lanes: recip(1)=1, no edge cases

            nc.sync.dma_start(bias_sb[:], bias[:])
            for fc in range(2):
                nc.sync.dma_start(wo_sb[fc][:], woT[fc * P : (fc + 1) * P, :])
            # memset/affine_select can't encode f32r/bf16 targets: build f32, cast
            nc.gpsimd.memset(onesf[:], 1.0)
            for t in range(2):
                nc.vector.tensor_copy(
                    ones33[32 * t : 32 * t + 1, :], onesf[0:1, :]
                )
            # keep elements where j (partition) <= i (free): upper tri incl diag
            make_upper_triangular(nc, trimaskf[:], val=1.0, diag=True)
            nc.vector.tensor_copy(trimask[:], trimaskf[:])
            # ones/pad columns of von (Z rows): cols [64:66] of each 66-chunk
            for h in range(HC):
                dst = von[:].rearrange("p (g n t) -> p g n t", g=HC, t=66)[
                    :, h, :, 64:66
                ]
                nc.vector.tensor_copy(
                    dst, onesf[:, 0:32].rearrange("p (n t) -> p n t", t=2)
                )

            # ---------------- Phase 1: QKV projection ----------------
            with (
                tc.tile_pool(name="xw", bufs=1) as xp,
                tc.tile_pool(name="p1ps", bufs=4, space="PSUM") as p1,
            ):
                x_sb = [
                    xp.tile([P, L], bf16, tag=f"x{ec}", name=f"x{ec}")
                    for ec in range(EC)
                ]
                wa_sb = [
                    xp.tile([P, 3 * F], bf16, tag=f"wa{ec}", name=f"wa{ec}")
                    for ec in range(EC)
                ]
                for ec in range(EC):
                    nc.sync.dma_start(x_sb[ec][:], xT[ec * P : (ec + 1) * P, :])
                    nc.sync.dma_start(wa_sb[ec][:], waT[ec * P : (ec + 1) * P, :])

                for lb in range(2):  # l-halves of 1024
                    ls = slice(lb * 1024, (lb + 1) * 1024)
                    for p in range(2):
                        ps_q = p1.tile([P, 1024], f32, tag="ps")
                        for ec in range(EC):
                            for s in range(2):  # psum bank limit: N <= 512
                                nc.tensor.matmul(
                                    ps_q[:, s * 512 : (s + 1) * 512],
                                    wa_sb[ec][:, p * P : (p + 1) * P],
                                    x_sb[ec][:, lb * 1024 + s * 512 : lb * 1024 + (s + 1) * 512],
                                    start=(ec == 0),
                                    stop=(ec == EC - 1),
                                )
                        nc.any.tensor_copy(qT[p][:, ls], ps_q[:])
                        ps_k = p1.tile([P, 1024], f32, tag="ps")
                        for ec in range(EC):
                            for s in range(2):
                                nc.tensor.matmul(
                                    ps_k[:, s * 512 : (s + 1) * 512],
                                    wa_sb[ec][:, 256 + p * P : 256 + (p + 1) * P],
                                    x_sb[ec][:, lb * 1024 + s * 512 : lb * 1024 + (s + 1) * 512],
                                    start=(ec == 0),
                                    stop=(ec == EC - 1),
                                )
                        nc.any.tensor_copy(kT[p][:, ls], ps_k[:])
                    # v natural [l, d] for all 4 heads at once (free dim 256)
                    for lc8 in range(8):
                        lc = lb * 8 + lc8
                        ps_v = p1.tile([P, F], f32, tag="ps")
                        for ec in range(EC):
                            nc.tensor.matmul(
                                ps_v[:],
                                x_sb[ec][:, lc * P : (lc + 1) * P],
                                wa_sb[ec][:, 2 * F : 3 * F],
                                start=(ec == 0),
                                stop=(ec == EC - 1),
                            )
                        # scatter the 4 heads' [128, 64] into von (cast to bf16)
                        dst = von[:].rearrange("p (g c) -> p g c", g=HC)[
                            :, :, lc * 66 : lc * 66 + 64
                        ]
                        src = ps_v[:].rearrange("p (g c) -> p g c", g=HC)
                        nc.any.tensor_copy(dst, src)

            # ------- Phase 2+3: causal attention + output projection -------
            # The two heads of a pair are interleaved unit-by-unit: their
            # scores matmuls live in disjoint PE row groups (partitions 0-63
            # vs 64-127), so LDWEIGHTS pull-ahead and concurrent row-group
            # matmuls both engage, and the PE queue always has independent
            # work (keeps the HAM clock-gate warm). PSUM fits by sweeping i
            # in two 1024-wide windows (causality completes Z per window).
            # Software-pipelining: the AV matmul of unit n-1 is emitted after
            # scores/exp of unit n; normalization and the output projection
            # of a finished pair are drip-fed into the next pair's stream.
            with (
                tc.tile_pool(name="sps", bufs=2, space="PSUM") as sp,
                tc.tile_pool(name="ops", bufs=2, space="PSUM") as op_,
                tc.tile_pool(name="epool", bufs=3) as ep,
                tc.tile_pool(name="npool", bufs=4) as npl,
                tc.tile_pool(name="ob", bufs=3) as ob,
            ):

                def emit_av(u):
                    h, po, et, jc, a0, w, win0 = u
                    for q in range((a0 - win0) // 512, (a0 - win0 + w - 1) // 512 + 1):
                        a = max(a0 - win0, q * 512)
                        bnd = min(a0 - win0 + w, (q + 1) * 512)
                        qg = win0 // 512 + q  # global 512-block of i
                        nc.tensor.matmul(
                            po[:, a:bnd],
                            von[:, h * VST + jc * 66 : h * VST + (jc + 1) * 66],
                            et[:, a - (a0 - win0) : bnd - (a0 - win0)],
                            start=(jc == 0),
                            stop=(jc == 4 * qg + 3),
                        )

                def emit_norm(p, win0, po2):
                    # rows 0..63 of po are o^T, row 64 is Z. Z rows land on
                    # partitions 0 and 32 (legal slice bases).
                    for hl in range(2):
                        nc.vector.tensor_copy(
                            zc[32 * hl : 32 * hl + 1, :], po2[hl][64:65, :]
                        )
                    zf = npl.tile([33, 1024], f32, tag="zf")
                    nc.vector.reciprocal_approx_fast(zf[:], zc[:])  # 1/Z
                    nc.vector.tensor_copy(zr[:], zf[:])  # cast to f32r
                    for hl in range(2):
                        zrep = sp.tile([64, 1024], f32, tag="ps")
                        for s in range(2):
                            nc.tensor.matmul(
                                zrep[:, s * 512 : (s + 1) * 512],
                                ones33[32 * hl : 32 * hl + 1, :],
                                zr[32 * hl : 32 * hl + 1, s * 512 : (s + 1) * 512],
                                start=True,
                                stop=True,
                            )
                        # TensorTensor can't take two PSUM inputs: bounce zrep
                        zs = npl.tile([64, 1024], f32, tag="zs")
                        nc.any.tensor_copy(zs[:], zrep[:])
                        nc.vector.tensor_mul(
                            oT[p][hl * 64 : (hl + 1) * 64, win0 : win0 + 1024],
                            po2[hl][0:64, :],
                            zs[:],
                        )

                def emit_oproj(fc, oc, lb):
                    # partial output projection for f-chunk fc (head pair fc);
                    # fc==0 writes (with bias), fc==1 DMA-accumulates.
                    ls = slice(lb * 1024, (lb + 1) * 1024)
                    ps = sp.tile([P, 1024], f32, tag="ps")
                    for s in range(2):
                        nc.tensor.matmul(
                            ps[:, s * 512 : (s + 1) * 512],
                            wo_sb[fc][:, oc * P : (oc + 1) * P],
                            oT[fc][:, lb * 1024 + s * 512 : lb * 1024 + (s + 1) * 512],
                            start=True,
                            stop=True,
                        )
                    ot = ob.tile([P, 1024], f32, tag="ot")
                    if fc == 0:
                        nc.vector.tensor_scalar_add(
                            ot[:], ps[:], bias_sb[:, oc : oc + 1]
                        )
                        nc.sync.dma_start(outT[oc * P : (oc + 1) * P, ls], ot[:])
                    else:
                        nc.any.tensor_copy(ot[:], ps[:])
                        nc.gpsimd.dma_start(
                            outT[oc * P : (oc + 1) * P, ls],
                            ot[:],
                            accum_op=mybir.AluOpType.add,
                        )

                pending_av = None
                filler = []  # deferred norm/out-proj units, drip-fed
                for p in range(2):  # head pair
                    for sweep in range(2):  # i-window [1024*sweep, +1024)
                        win0 = 1024 * sweep
                        po2 = [
                            op_.tile([66, 1024], f32, tag="po", name="po")
                            for _ in range(2)
                        ]
                        for jc in range((win0 + 1024) // P):
                            j0 = jc * P
                            a0 = max(j0, win0)
                            w = win0 + 1024 - a0
                            for hl in range(2):
                                h = 2 * p + hl
                                hp = slice(hl * 64, (hl + 1) * 64)
                                ps = sp.tile([P, 1024], f32, tag="ps")
                                for s0 in range(0, w, 512):
                                    sw = min(512, w - s0)
                                    nc.tensor.matmul(
                                        ps[:, s0 : s0 + sw],
                                        kT[p][hp, j0 : j0 + P],
                                        qT[p][hp, a0 + s0 : a0 + s0 + sw],
                                        start=True,
                                        stop=True,
                                    )
                                et = ep.tile([P, 1024], bf16, tag="e")
                                nc.scalar.activation(
                                    et[:, :w], ps[:, :w], AF.Exp, scale=0.125
                                )
                                if a0 == j0:
                                    # diagonal block: zero where j > i
                                    nc.vector.tensor_mul(
                                        et[:, :P], et[:, :P], trimask[:]
                                    )
                                if pending_av is not None:
                                    emit_av(pending_av)
                                pending_av = (h, po2[hl], et, jc, a0, w, win0)
                                if filler:
                                    filler.pop(0)()
                        filler.append(
                            lambda p=p, win0=win0, po2=po2: emit_norm(p, win0, po2)
                        )
                    # pair p done (after its pending norms run): queue its
                    # output-projection pass to interleave with the next pair
                    for oc in range(E // P):
                        for lb in range(2):
                            filler.append(
                                lambda fc=p, oc=oc, lb=lb: emit_oproj(fc, oc, lb)
                            )
                emit_av(pending_av)
                for f in filler:
                    f()

    nc.compile()
    return nc


def make_in_maps(x, Wa, Wout_w, Wout_b):
    """Host-side sharding: per-core input dicts."""
    x = np.asarray(x, dtype=np.float32)
    Wa = np.asarray(Wa, dtype=np.float32)
    Wout_w = np.asarray(Wout_w, dtype=np.float32)
    Wout_b = np.asarray(Wout_b, dtype=np.float32)
    b16 = ml_dtypes.bfloat16

    xTs = [np.ascontiguousarray(x[b].T).astype(b16) for b in range(B)]
    in_maps = []
    for c in range(N_CORES):
        b, hg = divmod(c, 4)
        heads = list(range(4 * hg, 4 * hg + 4))
        qrows = np.concatenate([Wa[192 * h : 192 * h + 64] for h in heads], 0)
        krows = np.concatenate([Wa[192 * h + 64 : 192 * h + 128] for h in heads], 0)
        vrows = np.concatenate([Wa[192 * h + 128 : 192 * h + 192] for h in heads], 0)
        waT = np.ascontiguousarray(
            np.concatenate([qrows, krows, vrows], 0).T
        ).astype(b16)
        woT = np.ascontiguousarray(
            np.concatenate([Wout_w[:, 64 * h : 64 * h + 64] for h in heads], 1).T
        ).astype(b16)
        bvec = Wout_b if hg == 0 else np.zeros_like(Wout_b)
        bias2d = np.ascontiguousarray(bvec.reshape(E // P, P).T)
        in_maps.append({"xT": xTs[b], "waT": waT, "woT": woT, "bias": bias2d})
    return in_maps


def combine_outputs(core_outs):
    """core_outs: list of 8 outT [E, L] partials -> full [B, L, E]."""
    out = np.empty((B, L, E), np.float32)
    for b in range(B):
        acc = np.asarray(core_outs[4 * b], np.float32)
        for c in range(4 * b + 1, 4 * b + 4):
            acc = acc + np.asarray(core_outs[c], np.float32)
        out[b] = acc.T
    return out


def kernel(x, Wa, Wout_w, Wout_b):
    nc = build_nc()
    in_maps = make_in_maps(x, Wa, Wout_w, Wout_b)
    res = run_bass_kernel_spmd(nc, in_maps, list(range(N_CORES)))
    return combine_outputs([r["outT"] for r in res.results])


if __name__ == "__main__":
    rng = np.random.default_rng(0)
    x = rng.standard_normal((B, L, E), dtype=np.float32)
    Wa = rng.standard_normal((3 * H * D, E), dtype=np.float32) * 0.02
    Ww = rng.standard_normal((E, H * D), dtype=np.float32) * 0.02
    Wb = rng.standard_normal((E,), dtype=np.float32) * 0.02
    out = kernel(x, Wa=Wa, Wout_w=Ww, Wout_b=Wb)
    print(out.shape, out.dtype)



# revision 14
# speedup vs baseline: 1.2671x; 1.2671x over previous
"""Multi-head causal attention (B=2, L=2048, E=1024, H=16, D=64) on 8 NeuronCores.

Sharding: data-parallel over batch x tensor-parallel over heads.
  core c: batch b = c // 4, head group hg = c % 4 -> heads [4*hg, 4*hg+4).
Each core computes QKV projection for its 4 heads, causal softmax attention,
and a *partial* output projection (its heads' slice of Wout). The host sums
the 4 partial outputs per batch (bias is folded into core hg==0's partial).

Device notes:
  - Matmul operands are bf16 (fp32 PSUM accumulation); host pre-casts inputs.
  - Host pre-transposes everything so the device never transposes:
      xT   [E, L]   = x[b].T                      (bf16)
      waT  [E, 768] = Wa rows regrouped [q_h0..q_h3 | k_h0.. | v_h0..].T (bf16)
      woT  [256, E] = Wout_w columns for this core's heads, transposed   (bf16)
      bias [128, 8] = Wout_b per output-partition chunk (zeros unless hg==0)
  - Attention runs in the S^T layout (scores[j, i]); softmax denominator Z
    comes from a ones-column appended to V (PSUM row 64; row 65 is pad).
  - No max-subtraction in softmax: scores are ~N(0, 0.41^2), exp can't overflow.
  - 1/Z computed as exp(-ln(Z)) on ScalarE (DVE reciprocal is ~3.3us/op).
"""

import ml_dtypes
import numpy as np

import concourse.bass as bass
import concourse.mybir as mybir
import concourse.tile as tile
from concourse import bacc
from concourse import bass_utils as _bass_utils
from concourse.bass_utils import run_bass_kernel_spmd
from concourse.masks import make_upper_triangular


P = 128
B = 2
L = 2048
E = 1024
H = 16
D = 64
HC = 4            # heads per core
F = HC * D        # 256: this core's slice of the head dim
EC = E // P       # 8 chunks of the embed dim
NLC = L // P      # 16 l-chunks
VST = NLC * 66    # v stride per head: 16 chunks of [64 v | 1 ones | 1 pad]

f32 = mybir.dt.float32
f32r = mybir.dt.float32r
bf16 = mybir.dt.bfloat16
AF = mybir.ActivationFunctionType
N_CORES = 8


def build_nc():
    nc = bacc.Bacc(None, target_bir_lowering=False, debug=False)

    xT = nc.dram_tensor("xT", [E, L], bf16, kind="ExternalInput")
    waT = nc.dram_tensor("waT", [E, 3 * F], bf16, kind="ExternalInput")
    woT = nc.dram_tensor("woT", [F, E], bf16, kind="ExternalInput")
    bias = nc.dram_tensor("bias", [P, E // P], f32, kind="ExternalInput")
    # Per-pair partial outputs (bf16 halves the write traffic); host sums.
    outT = nc.dram_tensor("outT", [E, L], bf16, kind="ExternalOutput")
    outT1 = nc.dram_tensor("outT1", [E, L], bf16, kind="ExternalOutput")

    with tile.TileContext(nc) as tc:
        with (
            tc.tile_pool(name="persist", bufs=1) as pp,
            tc.tile_pool(name="qkv", bufs=1) as qp,
        ):
            # Persistent SBUF tensors.
            qT = [qp.tile([P, L], bf16, tag=f"q{p}", name=f"qT{p}") for p in range(2)]
            kT = [qp.tile([P, L], bf16, tag=f"k{p}", name=f"kT{p}") for p in range(2)]
            von = qp.tile([P, HC * VST], bf16, tag="von", name="von")
            oT = [qp.tile([P, L], bf16, tag=f"o{p}", name=f"oT{p}") for p in range(2)]
            wo_sb = [
                pp.tile([P, E], bf16, tag=f"wo{fc}", name=f"wo{fc}") for fc in range(2)
            ]
            bias_sb = pp.tile([P, E // P], f32, tag="bias")
            trimask = pp.tile([P, P], bf16, tag="trimask")
            onesf = pp.tile([P, 64], f32, tag="onesf")
            trimaskf = pp.tile([P, P], f32, tag="trimaskf")

            nc.sync.dma_start(bias_sb[:], bias[:])
            for fc in range(2):
                nc.sync.dma_start(wo_sb[fc][:], woT[fc * P : (fc + 1) * P, :])
            # memset/affine_select can't encode f32r/bf16 targets: build f32, cast
            nc.gpsimd.memset(onesf[:], 1.0)
            # keep elements where j (partition) <= i (free): upper tri incl diag
            make_upper_triangular(nc, trimaskf[:], val=1.0, diag=True)
            nc.vector.tensor_copy(trimask[:], trimaskf[:])
            # ones/pad columns of von (Z rows): cols [64:66] of each 66-chunk
            for h in range(HC):
                dst = von[:].rearrange("p (g n t) -> p g n t", g=HC, t=66)[
                    :, h, :, 64:66
                ]
                nc.vector.tensor_copy(
                    dst, onesf[:, 0:32].rearrange("p (n t) -> p n t", t=2)
                )

            # ---------------- Phase 1: QKV projection ----------------
            with (
                tc.tile_pool(name="xw", bufs=1) as xp,
                tc.tile_pool(name="p1ps", bufs=4, space="PSUM") as p1,
            ):
                x_sb = [
                    xp.tile([P, L], bf16, tag=f"x{ec}", name=f"x{ec}")
                    for ec in range(EC)
                ]
                wa_sb = [
                    xp.tile([P, 3 * F], bf16, tag=f"wa{ec}", name=f"wa{ec}")
                    for ec in range(EC)
                ]
                for ec in range(EC):
                    nc.sync.dma_start(x_sb[ec][:], xT[ec * P : (ec + 1) * P, :])
                    nc.sync.dma_start(wa_sb[ec][:], waT[ec * P : (ec + 1) * P, :])

                for lb in range(2):  # l-halves of 1024
                    ls = slice(lb * 1024, (lb + 1) * 1024)
                    for p in range(2):
                        ps_q = p1.tile([P, 1024], f32, tag="ps")
                        for ec in range(EC):
                            for s in range(2):  # psum bank limit: N <= 512
                                nc.tensor.matmul(
                                    ps_q[:, s * 512 : (s + 1) * 512],
                                    wa_sb[ec][:, p * P : (p + 1) * P],
                                    x_sb[ec][:, lb * 1024 + s * 512 : lb * 1024 + (s + 1) * 512],
                                    start=(ec == 0),
                                    stop=(ec == EC - 1),
                                )
                        nc.vector.tensor_copy(qT[p][:, ls], ps_q[:])
                        ps_k = p1.tile([P, 1024], f32, tag="ps")
                        for ec in range(EC):
                            for s in range(2):
                                nc.tensor.matmul(
                                    ps_k[:, s * 512 : (s + 1) * 512],
                                    wa_sb[ec][:, 256 + p * P : 256 + (p + 1) * P],
                                    x_sb[ec][:, lb * 1024 + s * 512 : lb * 1024 + (s + 1) * 512],
                                    start=(ec == 0),
                                    stop=(ec == EC - 1),
                                )
                        nc.vector.tensor_copy(kT[p][:, ls], ps_k[:])
                    # v natural [l, d] for all 4 heads at once (free dim 256)
                    for lc8 in range(8):
                        lc = lb * 8 + lc8
                        ps_v = p1.tile([P, F], f32, tag="ps")
                        for ec in range(EC):
                            nc.tensor.matmul(
                                ps_v[:],
                                x_sb[ec][:, lc * P : (lc + 1) * P],
                                wa_sb[ec][:, 2 * F : 3 * F],
                                start=(ec == 0),
                                stop=(ec == EC - 1),
                            )
                        # scatter the 4 heads' [128, 64] into von (cast to bf16)
                        dst = von[:].rearrange("p (g c) -> p g c", g=HC)[
                            :, :, lc * 66 : lc * 66 + 64
                        ]
                        src = ps_v[:].rearrange("p (g c) -> p g c", g=HC)
                        nc.vector.tensor_copy(dst, src)

            # ------- Phase 2+3: causal attention + output projection -------
            # The two heads of a pair are interleaved unit-by-unit: their
            # scores matmuls live in disjoint PE row groups (partitions 0-63
            # vs 64-127), so LDWEIGHTS pull-ahead and concurrent row-group
            # matmuls both engage, and the PE queue always has independent
            # work (keeps the HAM clock-gate warm). PSUM fits by sweeping i
            # in two 1024-wide windows (causality completes Z per window).
            # Software-pipelining: the AV matmul of unit n-1 is emitted after
            # scores/exp of unit n; normalization and the output projection
            # of a finished pair are drip-fed into the next pair's stream.
            with (
                tc.tile_pool(name="sps", bufs=2, space="PSUM") as sp,
                tc.tile_pool(name="ops", bufs=2, space="PSUM") as op_,
                tc.tile_pool(name="epool", bufs=4) as ep,
                tc.tile_pool(name="npool", bufs=4) as npl,
                tc.tile_pool(name="ob", bufs=3) as ob,
            ):

                def emit_av(u):
                    h, po, et, jc, a0, w, win0 = u
                    for q in range((a0 - win0) // 512, (a0 - win0 + w - 1) // 512 + 1):
                        a = max(a0 - win0, q * 512)
                        bnd = min(a0 - win0 + w, (q + 1) * 512)
                        qg = win0 // 512 + q  # global 512-block of i
                        nc.tensor.matmul(
                            po[:, a:bnd],
                            von[:, h * VST + jc * 66 : h * VST + (jc + 1) * 66],
                            et[:, a - (a0 - win0) : bnd - (a0 - win0)],
                            start=(jc == 0),
                            stop=(jc == 4 * qg + 3),
                        )

                def emit_norm(p, win0, po2):
                    # rows 0..63 of po are o^T, row 64 is Z. Both heads' Z
                    # rows land side by side on partition 0 (the only
                    # partition GpSimd partition_broadcast can source from).
                    zc = npl.tile([1, 2048], f32, tag="zc")
                    for hl in range(2):
                        nc.vector.tensor_copy(
                            zc[0:1, 1024 * hl : 1024 * (hl + 1)], po2[hl][64:65, :]
                        )
                    zf = npl.tile([1, 2048], f32, tag="zf")
                    nc.vector.reciprocal_approx_fast(zf[:], zc[:])  # 1/Z
                    for hl in range(2):
                        # replicate 1/Z across the 64 o^T partitions (GpSimd;
                        # keeps PE and PSUM out of the normalization path)
                        zs = npl.tile([64, 1024], f32, tag="zs")
                        nc.gpsimd.partition_broadcast(
                            zs[:], zf[0:1, 1024 * hl : 1024 * (hl + 1)]
                        )
                        nc.vector.tensor_mul(
                            oT[p][hl * 64 : (hl + 1) * 64, win0 : win0 + 1024],
                            po2[hl][0:64, :],
                            zs[:],
                        )

                def emit_oproj(fc, oc, lb):
                    # partial output projection for f-chunk fc (head pair fc);
                    # each pair writes its own bf16 partial; the host sums.
                    ls = slice(lb * 1024, (lb + 1) * 1024)
                    ps = sp.tile([P, 1024], f32, tag="ps")
                    for s in range(2):
                        nc.tensor.matmul(
                            ps[:, s * 512 : (s + 1) * 512],
                            wo_sb[fc][:, oc * P : (oc + 1) * P],
                            oT[fc][:, lb * 1024 + s * 512 : lb * 1024 + (s + 1) * 512],
                            start=True,
                            stop=True,
                        )
                    ot = ob.tile([P, 1024], bf16, tag="ot")
                    if fc == 0:
                        nc.vector.tensor_scalar_add(
                            ot[:], ps[:], bias_sb[:, oc : oc + 1]
                        )
                        nc.sync.dma_start(outT[oc * P : (oc + 1) * P, ls], ot[:])
                    else:
                        nc.vector.tensor_copy(ot[:], ps[:])
                        nc.sync.dma_start(outT1[oc * P : (oc + 1) * P, ls], ot[:])

                pending_av = None
                filler = []  # deferred norm/out-proj units, drip-fed
                for p in range(2):  # head pair
                    for sweep in range(2):  # i-window [1024*sweep, +1024)
                        win0 = 1024 * sweep
                        po2 = [
                            op_.tile([66, 1024], f32, tag="po", name="po")
                            for _ in range(2)
                        ]
                        for jc in range((win0 + 1024) // P):
                            j0 = jc * P
                            a0 = max(j0, win0)
                            w = win0 + 1024 - a0
                            for hl in range(2):
                                h = 2 * p + hl
                                hp = slice(hl * 64, (hl + 1) * 64)
                                ps = sp.tile([P, 1024], f32, tag="ps")
                                for s0 in range(0, w, 512):
                                    sw = min(512, w - s0)
                                    nc.tensor.matmul(
                                        ps[:, s0 : s0 + sw],
                                        kT[p][hp, j0 : j0 + P],
                                        qT[p][hp, a0 + s0 : a0 + s0 + sw],
                                        start=True,
                                        stop=True,
                                    )
                                et = ep.tile([P, 1024], bf16, tag="e")
                                nc.scalar.activation(
                                    et[:, :w], ps[:, :w], AF.Exp, scale=0.125
                                )
                                if a0 == j0:
                                    # diagonal block: zero where j > i
                                    nc.vector.tensor_mul(
                                        et[:, :P], et[:, :P], trimask[:]
                                    )
                                if pending_av is not None:
                                    emit_av(pending_av)
                                pending_av = (h, po2[hl], et, jc, a0, w, win0)
                                if filler:
                                    filler.pop(0)()
                        filler.append(
                            lambda p=p, win0=win0, po2=po2: emit_norm(p, win0, po2)
                        )
                        # this window's l-half of oT is final once its norm
                        # runs: queue its output projection right behind it
                        for oc in range(E // P):
                            filler.append(
                                lambda fc=p, oc=oc, lb=sweep: emit_oproj(fc, oc, lb)
                            )
                emit_av(pending_av)
                for f in filler:
                    f()

    nc.compile()
    return nc


def make_in_maps(x, Wa, Wout_w, Wout_b):
    """Host-side sharding: per-core input dicts."""
    x = np.asarray(x, dtype=np.float32)
    Wa = np.asarray(Wa, dtype=np.float32)
    Wout_w = np.asarray(Wout_w, dtype=np.float32)
    Wout_b = np.asarray(Wout_b, dtype=np.float32)
    b16 = ml_dtypes.bfloat16

    xTs = [np.ascontiguousarray(x[b].T).astype(b16) for b in range(B)]
    in_maps = []
    for c in range(N_CORES):
        b, hg = divmod(c, 4)
        heads = list(range(4 * hg, 4 * hg + 4))
        qrows = np.concatenate([Wa[192 * h : 192 * h + 64] for h in heads], 0)
        krows = np.concatenate([Wa[192 * h + 64 : 192 * h + 128] for h in heads], 0)
        vrows = np.concatenate([Wa[192 * h + 128 : 192 * h + 192] for h in heads], 0)
        waT = np.ascontiguousarray(
            np.concatenate([qrows, krows, vrows], 0).T
        ).astype(b16)
        woT = np.ascontiguousarray(
            np.concatenate([Wout_w[:, 64 * h : 64 * h + 64] for h in heads], 1).T
        ).astype(b16)
        bvec = Wout_b if hg == 0 else np.zeros_like(Wout_b)
        bias2d = np.ascontiguousarray(bvec.reshape(E // P, P).T)
        in_maps.append({"xT": xTs[b], "waT": waT, "woT": woT, "bias": bias2d})
    return in_maps


def combine_outputs(core_outs):
    """core_outs: list of 8 (outT, outT1) [E, L] partials -> full [B, L, E]."""
    out = np.empty((B, L, E), np.float32)
    for b in range(B):
        acc = np.zeros((E, L), np.float32)
        for c in range(4 * b, 4 * b + 4):
            acc += np.asarray(core_outs[c][0], np.float32)
            acc += np.asarray(core_outs[c][1], np.float32)
        out[b] = acc.T
    return out


def kernel(x, Wa, Wout_w, Wout_b):
    nc = build_nc()
    in_maps = make_in_maps(x, Wa, Wout_w, Wout_b)
    res = run_bass_kernel_spmd(nc, in_maps, list(range(N_CORES)))
    return combine_outputs([(r["outT"], r["outT1"]) for r in res.results])


if __name__ == "__main__":
    rng = np.random.default_rng(0)
    x = rng.standard_normal((B, L, E), dtype=np.float32)
    Wa = rng.standard_normal((3 * H * D, E), dtype=np.float32) * 0.02
    Ww = rng.standard_normal((E, H * D), dtype=np.float32) * 0.02
    Wb = rng.standard_normal((E,), dtype=np.float32) * 0.02
    out = kernel(x, Wa=Wa, Wout_w=Ww, Wout_b=Wb)
    print(out.shape, out.dtype)



# revision 15
# speedup vs baseline: 1.4896x; 1.1756x over previous
"""Multi-head causal attention (B=2, L=2048, E=1024, H=16, D=64) on 8 NeuronCores.

Sharding: data-parallel over batch x tensor-parallel over heads.
  core c: batch b = c // 4, head group hg = c % 4 -> heads [4*hg, 4*hg+4).
Each core computes QKV projection for its 4 heads, causal softmax attention,
and per-head-pair partial output projections (pair0 partial carries the bias
on hg==0 cores). The host sums the 8 partials per batch.

Device structure (PE-density driven — keeps the HAM clock-gate warm):
  - QKV for pair 0 runs up front; QKV for pair 1 + normalization + output
    projection chunks are drip-fed as PE filler between attention units, so
    the tensor engine never idles while ScalarE chews on exp.
  - Attention runs in the S^T layout (scores[j, i]) over 512-wide i-windows;
    the two heads of a pair occupy disjoint PE row groups (partitions 0-63 /
    64-127) so their score matmuls run concurrently, and share one PSUM
    scores tile so each unit needs a single (strided) exp ACTIVATE.
  - Softmax denominator Z comes from a ones-column appended to V (PSUM row
    64 of the AV accumulator; row 65 is pad). No max-subtraction: scores are
    ~N(0, 0.41^2), exp can't overflow.
  - 1/Z on DVE (reciprocal_approx_fast), replicated across partitions with
    GpSimd partition_broadcast (source must be partition 0 — HW quirk).
  - Matmul operands are bf16 (fp32 PSUM accumulation); host pre-transposes
    everything so the device never transposes:
      xT   [E, L]   = x[b].T                      (bf16)
      waT  [E, 768] = Wa rows regrouped [q_h0..q_h3 | k_h0.. | v_h0..].T
      woT  [256, E] = Wout_w columns for this core's heads, transposed
      bias [128, 8] = Wout_b per output-partition chunk (zeros unless hg==0)
"""

import ml_dtypes
import numpy as np

import concourse.bass as bass
import concourse.mybir as mybir
import concourse.tile as tile
from concourse import bacc
from concourse import bass_utils as _bass_utils
from concourse.bass_utils import run_bass_kernel_spmd
from concourse.masks import make_upper_triangular


P = 128
B = 2
L = 2048
E = 1024
H = 16
D = 64
HC = 4            # heads per core
F = HC * D        # 256: this core's slice of the head dim
EC = E // P       # 8 chunks of the embed dim
NLC = L // P      # 16 l-chunks
VST = NLC * 66    # v stride per head: 16 chunks of [64 v | 1 ones | 1 pad]
W = 512           # attention i-window width
NW = L // W       # 4 windows

f32 = mybir.dt.float32
bf16 = mybir.dt.bfloat16
AF = mybir.ActivationFunctionType
N_CORES = 8


def build_nc():
    nc = bacc.Bacc(None, target_bir_lowering=False, debug=False)

    xT = nc.dram_tensor("xT", [E, L], bf16, kind="ExternalInput")
    waT = nc.dram_tensor("waT", [E, 3 * F], bf16, kind="ExternalInput")
    woT = nc.dram_tensor("woT", [F, E], bf16, kind="ExternalInput")
    bias = nc.dram_tensor("bias", [P, E // P], f32, kind="ExternalInput")
    # Per-pair partial outputs (bf16 halves the write traffic); host sums.
    outT = nc.dram_tensor("outT", [E, L], bf16, kind="ExternalOutput")
    outT1 = nc.dram_tensor("outT1", [E, L], bf16, kind="ExternalOutput")

    with tile.TileContext(nc) as tc:
        with (
            tc.tile_pool(name="persist", bufs=1) as pp,
            tc.tile_pool(name="qkv", bufs=1) as qp,
        ):
            # Persistent SBUF tensors.
            qT = [qp.tile([P, L], bf16, tag=f"q{p}", name=f"qT{p}") for p in range(2)]
            kT = [qp.tile([P, L], bf16, tag=f"k{p}", name=f"kT{p}") for p in range(2)]
            von = qp.tile([P, HC * VST], bf16, tag="von", name="von")
            oT = [qp.tile([P, L], bf16, tag=f"o{p}", name=f"oT{p}") for p in range(2)]
            x_sb = [
                qp.tile([P, L], bf16, tag=f"x{ec}", name=f"x{ec}") for ec in range(EC)
            ]
            wa_sb = [
                qp.tile([P, 3 * F], bf16, tag=f"wa{ec}", name=f"wa{ec}")
                for ec in range(EC)
            ]
            wo_sb = [
                pp.tile([P, E], bf16, tag=f"wo{fc}", name=f"wo{fc}") for fc in range(2)
            ]
            bias_sb = pp.tile([P, E // P], f32, tag="bias")
            trimask = pp.tile([P, P], bf16, tag="trimask")
            onesf = pp.tile([P, 64], f32, tag="onesf")
            trimaskf = pp.tile([P, P], f32, tag="trimaskf")

            nc.sync.dma_start(bias_sb[:], bias[:])
            for fc in range(2):
                nc.sync.dma_start(wo_sb[fc][:], woT[fc * P : (fc + 1) * P, :])
            for ec in range(EC):
                nc.sync.dma_start(x_sb[ec][:], xT[ec * P : (ec + 1) * P, :])
                nc.sync.dma_start(wa_sb[ec][:], waT[ec * P : (ec + 1) * P, :])
            # memset/affine_select can't encode bf16 targets: build f32, cast
            nc.gpsimd.memset(onesf[:], 1.0)
            # keep elements where j (partition) <= i (free): upper tri incl diag
            make_upper_triangular(nc, trimaskf[:], val=1.0, diag=True)
            nc.vector.tensor_copy(trimask[:], trimaskf[:])
            # ones/pad columns of von (Z rows): cols [64:66] of each 66-chunk
            for h in range(HC):
                dst = von[:].rearrange("p (g n t) -> p g n t", g=HC, t=66)[
                    :, h, :, 64:66
                ]
                nc.vector.tensor_copy(
                    dst, onesf[:, 0:32].rearrange("p (n t) -> p n t", t=2)
                )

            with (
                tc.tile_pool(name="sps", bufs=2, space="PSUM") as sp,
                tc.tile_pool(name="ops", bufs=2, space="PSUM") as op_,
                tc.tile_pool(name="fps", bufs=2, space="PSUM") as fp,
                tc.tile_pool(name="epool", bufs=4) as ep,
                tc.tile_pool(name="npool", bufs=4) as npl,
                tc.tile_pool(name="ob", bufs=3) as ob,
            ):

                def emit_qk_chunk(p, which, lb4):
                    # one 512-wide l-chunk of qT[p] (which=0) or kT[p] (=1)
                    ps = fp.tile([P, W], f32, tag="fps")
                    base = which * F + p * P
                    for ec in range(EC):
                        nc.tensor.matmul(
                            ps[:],
                            wa_sb[ec][:, base : base + P],
                            x_sb[ec][:, lb4 * W : (lb4 + 1) * W],
                            start=(ec == 0),
                            stop=(ec == EC - 1),
                        )
                    dst = (qT if which == 0 else kT)[p]
                    nc.vector.tensor_copy(dst[:, lb4 * W : (lb4 + 1) * W], ps[:])

                def emit_v_chunk(p, lc):
                    # v for pair p, l-chunk lc: [128 l, 2 heads x 64] natural
                    ps = fp.tile([P, P], f32, tag="fps")
                    for ec in range(EC):
                        nc.tensor.matmul(
                            ps[:],
                            x_sb[ec][:, lc * P : (lc + 1) * P],
                            wa_sb[ec][:, 2 * F + p * P : 2 * F + (p + 1) * P],
                            start=(ec == 0),
                            stop=(ec == EC - 1),
                        )
                    dst = von[:].rearrange("q (g n t) -> q g n t", g=HC, t=66)[
                        :, 2 * p : 2 * p + 2, lc, 0:64
                    ]
                    src = ps[:].rearrange("q (g c) -> q g c", g=2)
                    nc.vector.tensor_copy(dst, src)

                def emit_norm(p, win0, po2):
                    # rows 0..63 of po2 are o^T, row 64 is Z. Both heads' Z
                    # rows land side by side on partition 0 (the only
                    # partition GpSimd partition_broadcast can source from).
                    zc = npl.tile([1, 2 * W], f32, tag="zc")
                    for hl in range(2):
                        nc.vector.tensor_copy(
                            zc[0:1, W * hl : W * (hl + 1)], po2[hl][64:65, :]
                        )
                    zf = npl.tile([1, 2 * W], f32, tag="zf")
                    nc.vector.reciprocal_approx_fast(zf[:], zc[:])  # 1/Z
                    for hl in range(2):
                        zs = npl.tile([64, W], f32, tag="zs")
                        nc.gpsimd.partition_broadcast(
                            zs[:], zf[0:1, W * hl : W * (hl + 1)]
                        )
                        nc.vector.tensor_mul(
                            oT[p][hl * 64 : (hl + 1) * 64, win0 : win0 + W],
                            po2[hl][0:64, :],
                            zs[:],
                        )

                def emit_oproj(fc, oc, lb4):
                    # partial output projection for f-chunk fc (head pair fc);
                    # each pair writes its own bf16 partial; the host sums.
                    ls = slice(lb4 * W, (lb4 + 1) * W)
                    ps = fp.tile([P, W], f32, tag="fps")
                    nc.tensor.matmul(
                        ps[:],
                        wo_sb[fc][:, oc * P : (oc + 1) * P],
                        oT[fc][:, ls],
                        start=True,
                        stop=True,
                    )
                    ot = ob.tile([P, W], bf16, tag="ot")
                    if fc == 0:
                        nc.vector.tensor_scalar_add(
                            ot[:], ps[:], bias_sb[:, oc : oc + 1]
                        )
                        nc.sync.dma_start(outT[oc * P : (oc + 1) * P, ls], ot[:])
                    else:
                        nc.vector.tensor_copy(ot[:], ps[:])
                        nc.sync.dma_start(outT1[oc * P : (oc + 1) * P, ls], ot[:])

                filler = []  # deferred PE work, drip-fed between attn units

                def pop_fill():
                    if filler:
                        filler.pop(0)()

                # ---- Phase 1a: QKV for pair 0 (dense PE stream, warms HAM)
                for lb4 in range(NW):
                    emit_qk_chunk(0, 1, lb4)  # k first: scores read k @ jc=0
                    emit_qk_chunk(0, 0, lb4)
                for lc in range(NLC):
                    emit_v_chunk(0, lc)
                # QKV for pair 1 becomes filler inside pair-0 attention
                for lb4 in range(NW):
                    filler.append(lambda lb4=lb4: emit_qk_chunk(1, 1, lb4))
                    filler.append(lambda lb4=lb4: emit_qk_chunk(1, 0, lb4))
                for lc in range(NLC):
                    filler.append(lambda lc=lc: emit_v_chunk(1, lc))

                # ---- Phase 2: attention, fillers drip-fed per unit
                def emit_av(u):
                    p, po2, et, jc, a0, w, win0, njc = u
                    for hl in range(2):
                        h = 2 * p + hl
                        nc.tensor.matmul(
                            po2[hl][:, a0 - win0 : a0 - win0 + w],
                            von[:, h * VST + jc * 66 : h * VST + (jc + 1) * 66],
                            et[:, hl * W + (a0 - win0) : hl * W + (a0 - win0) + w],
                            start=(jc == 0),
                            stop=(jc == njc - 1),
                        )

                pending_av = None
                for p in range(2):  # head pair
                    for win in range(NW):  # i-window [W*win, +W)
                        win0 = W * win
                        po2 = [
                            op_.tile([66, W], f32, tag="po", name="po")
                            for _ in range(2)
                        ]
                        njc = (win0 + W) // P
                        for jc in range(njc):
                            j0 = jc * P
                            a0 = max(j0, win0)
                            w = win0 + W - a0
                            off = a0 - win0
                            ps = sp.tile([P, 2 * W], f32, tag="ps")
                            for hl in range(2):
                                hp = slice(hl * 64, (hl + 1) * 64)
                                nc.tensor.matmul(
                                    ps[:, hl * W + off : hl * W + off + w],
                                    kT[p][hp, j0 : j0 + P],
                                    qT[p][hp, a0 : a0 + w],
                                    start=True,
                                    stop=True,
                                )
                            et = ep.tile([P, 2 * W], bf16, tag="e")
                            pv = ps[:].rearrange("q (g c) -> q g c", g=2)[
                                :, :, off:W
                            ]
                            ev = et[:].rearrange("q (g c) -> q g c", g=2)[
                                :, :, off:W
                            ]
                            nc.scalar.activation(ev, pv, AF.Exp, scale=0.125)
                            if a0 == j0:
                                # diagonal block: zero where j > i
                                for hl in range(2):
                                    nc.vector.tensor_mul(
                                        et[:, hl * W + off : hl * W + off + P],
                                        et[:, hl * W + off : hl * W + off + P],
                                        trimask[:],
                                    )
                            if pending_av is not None:
                                emit_av(pending_av)
                            pending_av = (p, po2, et, jc, a0, w, win0, njc)
                            pop_fill()
                        filler.append(
                            lambda p=p, win0=win0, po2=po2: emit_norm(p, win0, po2)
                        )
                        # this window's l-chunk of oT is final once its norm
                        # runs: queue its output projection right behind it
                        for oc in range(E // P):
                            filler.append(
                                lambda fc=p, oc=oc, lb4=win: emit_oproj(fc, oc, lb4)
                            )
                emit_av(pending_av)
                for f in filler:
                    f()

    nc.compile()
    return nc


def make_in_maps(x, Wa, Wout_w, Wout_b):
    """Host-side sharding: per-core input dicts."""
    x = np.asarray(x, dtype=np.float32)
    Wa = np.asarray(Wa, dtype=np.float32)
    Wout_w = np.asarray(Wout_w, dtype=np.float32)
    Wout_b = np.asarray(Wout_b, dtype=np.float32)
    b16 = ml_dtypes.bfloat16

    xTs = [np.ascontiguousarray(x[b].T).astype(b16) for b in range(B)]
    in_maps = []
    for c in range(N_CORES):
        b, hg = divmod(c, 4)
        heads = list(range(4 * hg, 4 * hg + 4))
        qrows = np.concatenate([Wa[192 * h : 192 * h + 64] for h in heads], 0)
        krows = np.concatenate([Wa[192 * h + 64 : 192 * h + 128] for h in heads], 0)
        vrows = np.concatenate([Wa[192 * h + 128 : 192 * h + 192] for h in heads], 0)
        waT = np.ascontiguousarray(
            np.concatenate([qrows, krows, vrows], 0).T
        ).astype(b16)
        woT = np.ascontiguousarray(
            np.concatenate([Wout_w[:, 64 * h : 64 * h + 64] for h in heads], 1).T
        ).astype(b16)
        bvec = Wout_b if hg == 0 else np.zeros_like(Wout_b)
        bias2d = np.ascontiguousarray(bvec.reshape(E // P, P).T)
        in_maps.append({"xT": xTs[b], "waT": waT, "woT": woT, "bias": bias2d})
    return in_maps


def combine_outputs(core_outs):
    """core_outs: list of 8 (outT, outT1) [E, L] partials -> full [B, L, E]."""
    out = np.empty((B, L, E), np.float32)
    for b in range(B):
        acc = np.zeros((E, L), np.float32)
        for c in range(4 * b, 4 * b + 4):
            acc += np.asarray(core_outs[c][0], np.float32)
            acc += np.asarray(core_outs[c][1], np.float32)
        out[b] = acc.T
    return out


def kernel(x, Wa, Wout_w, Wout_b):
    nc = build_nc()
    in_maps = make_in_maps(x, Wa, Wout_w, Wout_b)
    res = run_bass_kernel_spmd(nc, in_maps, list(range(N_CORES)))
    return combine_outputs([(r["outT"], r["outT1"]) for r in res.results])


if __name__ == "__main__":
    rng = np.random.default_rng(0)
    x = rng.standard_normal((B, L, E), dtype=np.float32)
    Wa = rng.standard_normal((3 * H * D, E), dtype=np.float32) * 0.02
    Ww = rng.standard_normal((E, H * D), dtype=np.float32) * 0.02
    Wb = rng.standard_normal((E,), dtype=np.float32) * 0.02
    out = kernel(x, Wa=Wa, Wout_w=Ww, Wout_b=Wb)
    print(out.shape, out.dtype)
